# revision 1
# baseline (speedup 1.0000x reference)
"""Additive attention (tanh-score) via separable-basis approximation, TRN2 x8.

scores[b,q,k] = sum_h w_v[h] * tanh(qp[b,q,h] + kp[b,k,h])
              ~ sum_h w_v[h] * [ h(a) + sum_r f_r(qp) * g_r(kp) ]

The pure-a term h(a) is constant along k => softmax-invariant => dropped.
Each slot r: f_r, g_r are single ACT-evaluable atoms (sin/tanh/tanh2/poly with
affine args), so the (q,k,h) tanh tensor never materializes: the h-contraction
becomes PE matmuls with contraction dim H x R.

Sharding: 2 batches per core (big+small paired for balance, baked slot lens).
Per core everything is tiny: basis ACT passes over [128, ~1k] tiles, ~8R
matmuls of <=256 cols, fused-exp softmax, attn @ values.

Masking: one extra PE "slot" with lhsT=ones and rhs=mask rows (-8000 fp16,
x128 partitions => -1.024e6) => exp underflows to 0, matching the reference's
-1e6 mask through softmax.
"""

import os
import numpy as np
import ml_dtypes

_NCORES = 8
BF16 = ml_dtypes.bfloat16
FP16 = np.float16

# fit of tanh(a+b) ~ sum_r c * qatom(a) * katom(b)  (+ free pure-a term,
# dropped as softmax-invariant). atom (kind, w, s) -> func(w*x + s);
# kind 'poly' -> (x/3)**w. Fitted offline (Gaussian-weighted LS, wRMS 5.9e-3).
SLOTS = [
    (('sin', -2.5000000000, 0.2873262163),
     ('sin', -2.5000000000, -1.7679537402), 0.0454728641),
    (('sin', 1.5184027148, -3.2031135619),
     ('sin', -1.5172863264, -1.6228271159), 0.1612171955),
    (('sin', -0.4400441419, 1.5829518017),
     ('sin', -0.4404289683, -3.1577059831), 1.1875448050),
    (('sin', -2.4691279788, -1.3872222109),
     ('sin', 2.4692423942, 0.2646449817), -0.0516913971),
    (('sin', 1.3705130957, -1.6155418902),
     ('sin', 1.3700050256, 0.0533663718), -0.2331078125),
    (('sin', -0.7178178262, -3.1153805677),
     ('sin', -0.7182280272, 1.5504013584), 0.5396772381),
]


def _default_slots():
    import json
    env = os.environ.get("KERNEL_SLOTS_JSON", "")
    if env:
        return json.loads(env)
    cfgp = os.environ.get("KERNEL_SLOTS_FILE", "")
    if cfgp and os.path.exists(cfgp):
        return json.load(open(cfgp))
    return [(tuple(q), tuple(k), c) for q, k, c in SLOTS]


def _register_ntff_hook():
    import sys, types
    try:
        from antenv.axon_hooks import get_axon_ntff_profile_hook  # noqa: F401
        return
    except ImportError:
        pass
    try:
        import trn_agent_boot.trn_boot as tb
        mod = types.ModuleType("antenv.axon_hooks")
        hook = tb._ntff_profile_via_ctypes("/opt/axon/libaxon_pjrt.so")
        mod.get_axon_ntff_profile_hook = lambda: hook
        mod.set_axon_ntff_profile_hook = lambda h: None
        sys.modules["antenv.axon_hooks"] = mod
    except Exception:
        pass


NQ = 256
D = 256
H = 256
DV = 256
NDC = D // 128
NHC = H // 128
PS = 3.0  # poly atom normalization, matches fit


def _build_graph(LP0, LP1, slots):
    import concourse.bass as bass
    import concourse.tile as tile
    from concourse import bacc, mybir, masks

    f32 = mybir.dt.float32
    bf16 = mybir.dt.bfloat16
    fp16 = mybir.dt.float16
    i32 = mybir.dt.int32
    i16 = mybir.dt.int16
    AF = mybir.ActivationFunctionType
    ALU = mybir.AluOpType
    PSUM = bass.MemorySpace.PSUM

    LPT = LP0 + LP1
    LPs = (LP0, LP1)
    OFS = (0, LP0)
    NKC = ((LP0 + 127) // 128, (LP1 + 127) // 128)
    KCW = [[min(128, LPs[s] - 128 * c) for c in range(NKC[s])] for s in (0, 1)]

    AFMAP = {"tanh": AF.Tanh, "tanh2": AF.Tanh, "sin": AF.Sin, "square": AF.Square}

    nc = bacc.Bacc("TRN2", target_bir_lowering=False, debug=False,
                   num_devices=_NCORES)

    qT_d = nc.dram_tensor("qT", (128, NDC, 2, NQ), fp16, kind="ExternalInput")
    kT_d = nc.dram_tensor("kT", (128, NDC, LPT), fp16, kind="ExternalInput")
    W_d = nc.dram_tensor("Wst", (128, NDC, 2, H), fp16, kind="ExternalInput")
    wv_d = nc.dram_tensor("wvp", (128, NHC), f32, kind="ExternalInput")
    vals_d = nc.dram_tensor("valsp", (128, 4, DV), bf16, kind="ExternalInput")
    mask_d = nc.dram_tensor("maskv", (128, LPT), fp16, kind="ExternalInput")
    out_d = nc.dram_tensor("out", (2, NQ, DV), f32, kind="ExternalOutput")

    # q-side atoms: which need poly powers
    need_pow = sorted({int(qd[1]) for qd, kd, c in slots if qd[0] == "poly"})
    kneed_pow = sorted({int(kd[1]) for qd, kd, c in slots if kd[0] == "poly"})

    with tile.TileContext(nc) as tc:
        with (
            tc.tile_pool(name="const", bufs=1) as constp,
            tc.tile_pool(name="basis", bufs=1) as basisp,
            tc.tile_pool(name="work", bufs=2) as workp,
            tc.tile_pool(name="epi", bufs=1) as epip,
            tc.tile_pool(name="ppA", bufs=1, space=PSUM) as ppA,
            tc.tile_pool(name="ppS", bufs=1, space=PSUM) as ppS,
            tc.tile_pool(name="ppT", bufs=2, space=PSUM) as ppT,
        ):
            # ---- inputs / constants ----
            qT = constp.tile([128, NDC, 2, NQ], fp16)
            nc.sync.dma_start(qT[:], qT_d.ap())
            kT = constp.tile([128, NDC, LPT], fp16)
            nc.sync.dma_start(kT[:], kT_d.ap())
            Wst = constp.tile([128, NDC, 2, H], fp16)
            nc.sync.dma_start(Wst[:], W_d.ap())
            wv = constp.tile([128, NHC], f32)
            nc.sync.dma_start(wv[:], wv_d.ap())
            vals = constp.tile([128, 4, DV], bf16)
            nc.sync.dma_start(vals[:], vals_d.ap())
            maskv = constp.tile([128, LPT], fp16)
            nc.sync.dma_start(maskv[:], mask_d.ap())

            identf = constp.tile([128, 128], f32)
            masks.make_identity(nc, identf[:])
            ident_bf = constp.tile([128, 128], bf16)
            nc.vector.tensor_copy(ident_bf[:], identf[:])
            ones16 = constp.tile([128, 128], fp16)
            nc.vector.memset(ones16[:], 1.0)

            # per-partition bias tiles for activation() (floats need const APs)
            _bias_cache = {}

            def bias_ap(val):
                val = float(val)
                if val == 0.0:
                    return 0.0
                if val not in _bias_cache:
                    bt = constp.tile([128, 1], f32, name=f"bias{len(_bias_cache)}",
                                     tag=f"bias{len(_bias_cache)}")
                    nc.vector.memset(bt[:], val)
                    _bias_cache[val] = bt
                return _bias_cache[val][:]

            # ---- projections ----
            qp = ppA.tile([128, NHC, 2, NQ], f32, tag="qp")
            # kp padded to bank-aligned 512 f32 per hc region
            kp_full = ppA.tile([128, NHC, 512], f32, tag="kp")
            kp = kp_full[:, :, :LPT]
            for hc in range(NHC):
                for dc in range(NDC):
                    nc.tensor.matmul(
                        qp[:, hc], Wst[:, dc, 0, 128 * hc:128 * (hc + 1)],
                        qT[:, dc],
                        start=(dc == 0),
                        stop=(dc == NDC - 1),
                    )
            for hc in range(NHC):
                for dc in range(NDC):
                    nc.tensor.matmul(
                        kp_full[:, hc, :LPT], Wst[:, dc, 1, 128 * hc:128 * (hc + 1)],
                        kT[:, dc],
                        start=(dc == 0),
                        stop=(dc == NDC - 1),
                    )

            # qp/kp copied PSUM->SBUF once as fp16: contiguous, frees banks
            # early, and the 16-bit basis reduction passes run 2-4x on DVE.
            kp_sb = basisp.tile([128, NHC, LPT], fp16, tag="kp_sb")
            nc.vector.tensor_copy(kp_sb[:], kp)
            kp = kp_sb
            qp_sb = basisp.tile([128, NHC, 2, NQ], fp16, tag="qp_sb")
            nc.vector.tensor_copy(
                qp_sb[:].rearrange("p c s q -> p (c s q)"),
                qp[:].rearrange("p c s q -> p (c s q)"))
            qp = qp_sb

            TWO_PI = 6.283185307179586

            def emit_atom(dst, src, kind, w, s, shape_tag):
                """dst: fp16 tile AP (flattened), src: f32 SBUF AP (flattened).
                kind in {tanh, tanh2, sin, nsin}. Returns the value tile AP."""
                w = float(w); s = float(s)
                if kind in ("tanh", "tanh2", "nsin"):
                    func = AF.Sin if kind == "nsin" else AF.Tanh
                    nc.scalar.activation(dst, src, func,
                                         bias=bias_ap(s), scale=w)
                    if kind == "tanh2":
                        nc.vector.scalar_tensor_tensor(
                            dst, dst, 1.0, dst, ALU.mult, ALU.mult)
                    return
                assert kind == "sin"
                # Range reduction on DVE: n = round((wx+s)/2pi) via the
                # round-to-nearest fp->int output conversion, then
                # frac = wx/2pi - n; ACT computes sin(2pi*frac + s) with the
                # affine fused (final arg guaranteed inside the Sin spline's
                # [-pi, pi] domain).
                n_i16 = workp.tile(list(src.shape), i16, tag=f"n{shape_tag}",
                                   name=f"n{shape_tag}")
                nc.vector.tensor_scalar(
                    n_i16[:], src, w / TWO_PI, s / TWO_PI, ALU.mult, ALU.add)
                frac = workp.tile(list(src.shape), fp16, tag=f"fr{shape_tag}",
                                  name=f"fr{shape_tag}")
                nc.vector.scalar_tensor_tensor(
                    frac[:], src, w / TWO_PI, n_i16[:], ALU.mult, ALU.subtract)
                nc.scalar.activation(dst, frac[:], AF.Sin,
                                     bias=bias_ap(s), scale=TWO_PI)

            # ---- poly powers (shared) ----
            qpow = {}
            if need_pow:
                q1 = basisp.tile([128, NHC, 2, NQ], fp16, tag="q1")
                nc.vector.tensor_scalar(
                    q1[:].rearrange("p c s q -> p (c s q)"),
                    qp[:].rearrange("p c s q -> p (c s q)"),
                    1.0 / PS, None, ALU.mult)
                qpow[1] = q1
                if {2, 3} & set(need_pow):
                    q2 = basisp.tile([128, NHC, 2, NQ], fp16, tag="q2")
                    nc.scalar.activation(
                        q2[:].rearrange("p c s q -> p (c s q)"),
                        qp[:].rearrange("p c s q -> p (c s q)"),
                        AF.Square, scale=1.0 / PS)
                    qpow[2] = q2
                if 3 in need_pow:
                    q3 = basisp.tile([128, NHC, 2, NQ], fp16, tag="q3")
                    nc.vector.scalar_tensor_tensor(
                        q3[:].rearrange("p c s q -> p (c s q)"),
                        q1[:].rearrange("p c s q -> p (c s q)"), 1.0,
                        q2[:].rearrange("p c s q -> p (c s q)"),
                        ALU.mult, ALU.mult)
                    qpow[3] = q3
            kpow = {}
            if kneed_pow:
                k1 = basisp.tile([128, NHC, LPT], fp16, tag="k1")
                nc.vector.tensor_scalar(
                    k1[:].rearrange("p c k -> p (c k)"),
                    kp[:].rearrange("p c k -> p (c k)"),
                    1.0 / PS, None, ALU.mult)
                kpow[1] = k1
                if {2, 3} & set(kneed_pow):
                    k2 = basisp.tile([128, NHC, LPT], fp16, tag="k2")
                    nc.scalar.activation(
                        k2[:].rearrange("p c k -> p (c k)"),
                        kp[:].rearrange("p c k -> p (c k)"),
                        AF.Square, scale=1.0 / PS)
                    kpow[2] = k2
                if 3 in kneed_pow:
                    k3 = basisp.tile([128, NHC, LPT], fp16, tag="k3")
                    nc.vector.scalar_tensor_tensor(
                        k3[:].rearrange("p c k -> p (c k)"),
                        k1[:].rearrange("p c k -> p (c k)"), 1.0,
                        k2[:].rearrange("p c k -> p (c k)"),
                        ALU.mult, ALU.mult)
                    kpow[3] = k3

            sc = [ppS.tile([128, 2, LPs[s]], f32, tag=f"sc{s}", name=f"sc{s}")
                  for s in (0, 1)]
            nmm = 0

            def emit_slot_mms(A_tile, V_tile, first):
                nonlocal nmm
                for s in (0, 1):
                    for qc in range(2):
                        for hc in range(NHC):
                            if A_tile is None:  # poly j=0: ones lhsT
                                lhsT = ones16[:]
                            else:
                                lhsT = A_tile[:, hc, s, 128 * qc:128 * (qc + 1)]
                            nc.tensor.matmul(
                                sc[s][:, qc, :], lhsT,
                                V_tile[:, hc, OFS[s]:OFS[s] + LPs[s]],
                                start=(first and qc == 0 and hc == 0),
                                stop=False,
                            )
                            nmm += 1

            # ---- basis slots (software-pipelined: slot r's wv-fold + MMs
            # are emitted after slot r+1's reduction/ACT passes so the
            # in-order DVE stream never stalls waiting on ACT) ----
            def emit_basis(r, qd, kd):
                if kd[0] == "poly":
                    j = int(kd[1])
                    assert j >= 1, "pure-constant k atom is softmax-invariant"
                    G = kpow[j]
                else:
                    G = workp.tile([128, NHC, LPT], fp16, tag="G", name=f"G{r}")
                    emit_atom(G[:].rearrange("p c k -> p (c k)"),
                              kp[:].rearrange("p c k -> p (c k)"),
                              kd[0], kd[1], kd[2], "k")
                if qd[0] == "poly":
                    j = int(qd[1])
                    A = None if j == 0 else qpow[j]
                else:
                    A = basisp.tile([128, NHC, 2, NQ], fp16, tag=f"A{r}",
                                    name=f"A{r}")
                    emit_atom(A[:].rearrange("p c s q -> p (c s q)"),
                              qp[:].rearrange("p c s q -> p (c s q)"),
                              qd[0], qd[1], qd[2], "q")
                return A, G

            def emit_fold_mms(r, A, G, c):
                V = basisp.tile([128, NHC, LPT], fp16, tag=f"V{r}",
                                name=f"V{r}")
                for hc in range(NHC):
                    nc.vector.tensor_scalar(
                        V[:, hc], G[:, hc], wv[:, hc:hc + 1], float(c),
                        ALU.mult, ALU.mult)
                emit_slot_mms(A, V, first=(r == 0))

            pending = None
            for r, (qd, kd, c) in enumerate(slots):
                A, G = emit_basis(r, qd, kd)
                if pending is not None:
                    emit_fold_mms(*pending)
                pending = (r, A, G, c)
            emit_fold_mms(*pending)

            # ---- mask slot (stop=True closes the accumulation groups) ----
            for s in (0, 1):
                for qc in range(2):
                    nc.tensor.matmul(
                        sc[s][:, qc, :], ones16[:],
                        maskv[:, OFS[s]:OFS[s] + LPs[s]],
                        start=False, stop=(qc == 1),
                    )

            # ---- epilogue, both batch slots interleaved so PE/ACT/DVE
            # work on different slots concurrently; final scale on ACT
            # (Copy with per-partition rinv scale) to keep DVE free ----
            p_ts, rsums, rinvs, pTs, outps = {}, {}, {}, {}, {}
            for s in (0, 1):
                p_ts[s] = epip.tile([128, 2, LPs[s]], bf16, tag=f"pt{s}",
                                    name=f"pt{s}")
                rsums[s] = epip.tile([128, 2], f32, tag=f"rs{s}", name=f"rs{s}")
                for qc in range(2):
                    nc.scalar.activation(
                        p_ts[s][:, qc, :], sc[s][:, qc, :], AF.Exp,
                        accum_out=rsums[s][:, qc:qc + 1])
            for s in (0, 1):
                rinvs[s] = epip.tile([128, 2], f32, tag=f"ri{s}", name=f"ri{s}")
                nc.vector.reciprocal(rinvs[s][:], rsums[s][:])
            for s in (0, 1):
                pTs[s] = epip.tile([128, NKC[s], 2, 128], bf16, tag=f"pT{s}",
                                   name=f"pT{s}")
                for qc in range(2):
                    for kc in range(NKC[s]):
                        kw_ = KCW[s][kc]
                        tp = ppT.tile([128, 128], bf16, tag="tp")
                        nc.tensor.transpose(
                            tp[:kw_, :128],
                            p_ts[s][:, qc, 128 * kc:128 * kc + kw_],
                            ident_bf[:, :])
                        nc.vector.tensor_copy(pTs[s][:kw_, kc, qc],
                                              tp[:kw_, :128])
            for s in (0, 1):
                outp = ppS.tile([128, 2, DV], f32, tag=f"sc{s}", name=f"op{s}")
                outps[s] = outp
                for qc in range(2):
                    for kc in range(NKC[s]):
                        kw_ = KCW[s][kc]
                        nc.tensor.matmul(
                            outp[:, qc, :], pTs[s][:kw_, kc, qc],
                            vals[:kw_, 2 * s + kc, :],
                            start=(qc == 0 and kc == 0),
                            stop=(qc == 1 and kc == NKC[s] - 1),
                        )
            for s in (0, 1):
                out_sb = epip.tile([128, 2, DV], f32, tag=f"ob{s}",
                                   name=f"ob{s}")
                for qc in range(2):
                    nc.scalar.mul(out_sb[:, qc, :], outps[s][:, qc, :],
                                  rinvs[s][:, qc:qc + 1])
                    nc.sync.dma_start(
                        out_d.ap()[s, 128 * qc:128 * (qc + 1), :],
                        out_sb[:, qc, :])

    nc.compile()
    return nc


_GRAPH_CACHE = {}


def _get_graph(LP0, LP1, slots_key, slots):
    key = (LP0, LP1, slots_key)
    if key not in _GRAPH_CACHE:
        _GRAPH_CACHE[key] = _build_graph(LP0, LP1, slots)
    return _GRAPH_CACHE[key]


def kernel(queries, keys, values, valid_lens, W_q, W_k, w_v):
    from concourse import bass_utils

    queries = np.asarray(queries, dtype=np.float32)
    keys = np.asarray(keys, dtype=np.float32)
    values = np.asarray(values, dtype=np.float32)
    W_q = np.asarray(W_q, dtype=np.float32)
    W_k = np.asarray(W_k, dtype=np.float32)
    w_v = np.asarray(w_v, dtype=np.float32)
    vl = np.asarray(valid_lens).astype(np.int64)

    B = queries.shape[0]
    assert B == 2 * _NCORES

    slots_raw = _default_slots()
    slots = []
    for sdict in slots_raw:
        if isinstance(sdict, dict):
            q = sdict["q"]; k = sdict["k"]; c = sdict["c"]
        else:
            q, k, c = sdict
        qw = q[1] if q[1] is not None else 0.0
        qs = q[2] if q[2] is not None else 0.0
        kw = k[1] if k[1] is not None else 0.0
        ks = k[2] if k[2] is not None else 0.0
        slots.append(((q[0], float(qw), float(qs)),
                      (k[0], float(kw), float(ks)), float(c)))
    slots_key = str(slots)

    # pair batches: sort desc, pair i with (B-1-i)
    order = np.argsort(-vl, kind="stable")
    pairs = [(int(order[i]), int(order[B - 1 - i])) for i in range(_NCORES)]
    lv = lambda b: int(min(NQ, max(1, vl[b])))
    LP0 = max(-(-lv(b0) // 8) * 8 for b0, b1 in pairs)
    LP1 = max(-(-lv(b1) // 8) * 8 for b0, b1 in pairs)
    LPT = LP0 + LP1

    nc = _get_graph(LP0, LP1, slots_key, slots)

    # shared derived inputs
    def t128(x, rows):  # [rows, 256] -> [128, NDC, rows]
        xt = np.ascontiguousarray(x[:rows].T)          # [256, rows]
        return xt.reshape(NDC, 128, rows).transpose(1, 0, 2)

    Wst = np.ascontiguousarray(
        np.stack([W_q.reshape(NDC, 128, H).transpose(1, 0, 2),
                  W_k.reshape(NDC, 128, H).transpose(1, 0, 2)], axis=2)
    ).astype(FP16)  # [128, NDC, 2, H]
    wvp = np.ascontiguousarray(w_v.reshape(NHC, 128).T).astype(np.float32)

    in_maps = []
    for b0, b1 in pairs:
        qTa = np.empty((128, NDC, 2, NQ), np.float32)
        qTa[:, :, 0, :] = t128(queries[b0], NQ)
        qTa[:, :, 1, :] = t128(queries[b1], NQ)
        kTa = np.empty((128, NDC, LPT), np.float32)
        kTa[:, :, :LP0] = t128(keys[b0], LP0)
        kTa[:, :, LP0:] = t128(keys[b1], LP1)
        valsp = np.zeros((128, 4, DV), np.float32)
        for s, (b, LPs) in enumerate(((b0, LP0), (b1, LP1))):
            for kc in range(-(-LPs // 128)):
                kw_ = min(128, LPs - 128 * kc)
                valsp[:kw_, 2 * s + kc, :] = values[b, 128 * kc:128 * kc + kw_, :]
        maskv = np.zeros((128, LPT), np.float32)
        maskv[:, :LP0][:, np.arange(LP0) >= lv(b0)] = -8000.0
        maskv[:, LP0:][:, np.arange(LP1) >= lv(b1)] = -8000.0
        in_maps.append({
            "qT": np.ascontiguousarray(qTa).astype(FP16),
            "kT": np.ascontiguousarray(kTa).astype(FP16),
            "Wst": Wst,
            "wvp": wvp,
            "valsp": valsp.astype(BF16),
            "maskv": maskv.astype(FP16),
        })

    trace = os.environ.get("BASS_KERNEL_TRACE") == "1"
    if trace:
        _register_ntff_hook()
    res = bass_utils.run_bass_kernel_spmd(
        nc, in_maps, core_ids=list(range(_NCORES)), trace=trace)
    kernel.last_results = res

    out = np.empty((B, NQ, DV), dtype=np.float32)
    for j, (b0, b1) in enumerate(pairs):
        out[b0] = res.results[j]["out"][0]
        out[b1] = res.results[j]["out"][1]
    return out



# revision 3
# speedup vs baseline: 1.1008x; 1.1008x over previous
"""Additive attention (tanh-score) via separable sin-basis, TRN2 x8.

scores[b,q,k] = sum_h w_v[h] * tanh(qp[b,q,h] + kp[b,k,h])
              ~ sum_h w_v[h] * [ h(a) + sum_r c_r sin(w_r a + s_r)sin(w'_r b + s'_r) ]

Pure-a term is softmax-invariant => dropped. Each sin atom is evaluated as
  u = t - round(t),  t = (w x + s) / 2pi     (ONE fused custom DVE op,
                                              fp32 magic-constant rounding)
  atom = Sin(2pi * u)                        (ACT, scale/bias identical for
                                              every atom => q|k concatenated
                                              into one ACT call per slot)
The wv*c fold of the k-side runs on GpSimd (otherwise idle). Scores
accumulate in PSUM across slots; a mask pseudo-slot (ones lhsT x mask rows)
closes the groups. Softmax: fused-exp ACT with row-sum accumulator, PE
transposes, attn @ values on PE, 1/rowsum applied on the output tiles.

Sharding: 2 batches per core (big+small valid_len paired), baked LP0/LP1.
"""

import os
import numpy as np
import ml_dtypes

_NCORES = 8
BF16 = ml_dtypes.bfloat16
FP16 = np.float16

# (w_q, s_q, w_k, s_k, c): tanh(a+b) ~ sum c*sin(w_q a + s_q)*sin(w_k b + s_k)
SLOTS = [
    (-2.5000000000, 0.2873262163, -2.5000000000, -1.7679537402, 0.0454728641),
    (1.5184027148, -3.2031135619, -1.5172863264, -1.6228271159, 0.1612171955),
    (-0.4400441419, 1.5829518017, -0.4404289683, -3.1577059831, 1.1875448050),
    (-2.4691279788, -1.3872222109, 2.4692423942, 0.2646449817, -0.0516913971),
    (1.3705130957, -1.6155418902, 1.3700050256, 0.0533663718, -0.2331078125),
    (-0.7178178262, -3.1153805677, -0.7182280272, 1.5504013584, 0.5396772381),
]

NQ = 256
D = 256
H = 256
DV = 256
NDC = D // 128
NHC = H // 128
MAGIC = 12582912.0  # 1.5 * 2**23: fp32 add/sub rounds to nearest integer
TWO_PI = 6.283185307179586


def _register_ntff_hook():
    import sys, types
    try:
        from antenv.axon_hooks import get_axon_ntff_profile_hook  # noqa: F401
        return
    except ImportError:
        pass
    try:
        import trn_agent_boot.trn_boot as tb
        mod = types.ModuleType("antenv.axon_hooks")
        hook = tb._ntff_profile_via_ctypes("/opt/axon/libaxon_pjrt.so")
        mod.get_axon_ntff_profile_hook = lambda: hook
        mod.set_axon_ntff_profile_hook = lambda h: None
        sys.modules["antenv.axon_hooks"] = mod
    except Exception:
        pass


def _register_rr_op():
    """Custom DVE op: u = t - round(t), t = Src0*C0 + C1 (C2 = MAGIC)."""
    import concourse.dve_ops as dops
    from concourse.dve_spec import Spec, Src0, C0, C1, C2, lower, _has_src1
    from concourse.dve_uop import DveOpSpec

    for o in dops.OPS:
        if o.name == "SIN_RR_ANT":
            return o
    t = Src0 * C0 + C1
    n = (t + C2) - C2
    spec = Spec(
        body=t - n,
        reference=lambda in0, in1, s0, s1, imm2: (
            lambda tt: (tt - (np.float32(np.float32(tt) + np.float32(imm2))
                             - np.float32(imm2))).astype(np.float32)
        )(np.float32(in0) * np.float32(s0) + np.float32(s1)),
    )
    row = dops._CUSTOM_DVE_ROW_BASE + len(dops.OPS)
    assert row < 0x20
    shas = {}
    for ver in ("v3", "v4"):
        uops = lower(spec, ver=ver)
        s = DveOpSpec(name="SIN_RR_ANT", opcode=row, uops=uops,
                      rd1_en=_has_src1(spec))
        shas[ver] = s.sha(ver)
    op = dops.DveOp("SIN_RR_ANT", spec, subdim=False, uops_sha=shas)
    dops.OPS.append(op)
    dops._SUB_OPCODE_FOR_NAME[op.name] = row
    dops.CUSTOM_DVE_SPECS[op.name] = spec
    return op


def _build_graph(LP0, LP1):
    import concourse.bass as bass
    import concourse.tile as tile
    from concourse import bacc, mybir, masks

    RR = _register_rr_op()

    f32 = mybir.dt.float32
    bf16 = mybir.dt.bfloat16
    fp16 = mybir.dt.float16
    AF = mybir.ActivationFunctionType
    ALU = mybir.AluOpType
    PSUM = bass.MemorySpace.PSUM

    LPT = LP0 + LP1
    LPs = (LP0, LP1)
    OFS = (0, LP0)
    NKC = ((LP0 + 127) // 128, (LP1 + 127) // 128)
    KCW = [[min(128, LPs[s] - 128 * c) for c in range(NKC[s])] for s in (0, 1)]
    QW = 2 * NQ                  # 512 q-cols (both batch slots)
    W = QW + LPT                 # per-hc projection width

    nc = bacc.Bacc("TRN2", target_bir_lowering=False, debug=False,
                   num_devices=_NCORES)

    d1_d = nc.dram_tensor("wq_qT", (128, NDC, 256 + QW), fp16,
                          kind="ExternalInput")
    d2_d = nc.dram_tensor("wk_kT", (128, NDC, 256 + LPT), fp16,
                          kind="ExternalInput")
    d3_d = nc.dram_tensor("vm", (128, 4 * DV + LPT), fp16,
                          kind="ExternalInput")
    wv_d = nc.dram_tensor("wvp", (128, NHC), f32, kind="ExternalInput")
    out_d = nc.dram_tensor("out", (2, NQ, DV), bf16, kind="ExternalOutput")

    with tile.TileContext(nc) as tc:
        with (
            tc.tile_pool(name="const", bufs=1) as constp,
            tc.tile_pool(name="basis", bufs=1) as basisp,
            tc.tile_pool(name="uw", bufs=2) as uwp,
            tc.tile_pool(name="atw", bufs=2) as atwp,
            tc.tile_pool(name="vtw", bufs=2) as vtwp,
            tc.tile_pool(name="epi", bufs=1) as epip,
            tc.tile_pool(name="ppX", bufs=1, space=PSUM) as ppX,
            tc.tile_pool(name="ppS", bufs=1, space=PSUM) as ppS,
            tc.tile_pool(name="ppT", bufs=2, space=PSUM) as ppT,
        ):
            # ---- inputs (DMA order = need order) ----
            d1 = constp.tile([128, NDC, 256 + QW], fp16)
            nc.sync.dma_start(d1[:], d1_d.ap())
            d2 = constp.tile([128, NDC, 256 + LPT], fp16)
            nc.sync.dma_start(d2[:], d2_d.ap())
            d3 = constp.tile([128, 4 * DV + LPT], fp16)
            nc.sync.dma_start(d3[:], d3_d.ap())
            wv = constp.tile([128, NHC], f32)
            nc.sync.dma_start(wv[:], wv_d.ap())
            vals = d3[:, :4 * DV].rearrange("p (c v) -> p c v", c=4)
            maskv = d3[:, 4 * DV:]

            identf = constp.tile([128, 128], f32)
            masks.make_identity(nc, identf[:])
            ident_bf = constp.tile([128, 128], bf16)
            nc.vector.tensor_copy(ident_bf[:], identf[:])
            ones16 = constp.tile([128, 128], fp16)
            nc.vector.memset(ones16[:], 1.0)

            # ---- projections into PSUM: per hc [q-block 512 | k-block LPT]
            xcp = ppX.tile([128, NHC, 1024], f32, tag="xcp")
            for hc in range(NHC):
                for dc in range(NDC):
                    nc.tensor.matmul(
                        xcp[:, hc, 0:QW],
                        d1[:, dc, 128 * hc:128 * (hc + 1)],
                        d1[:, dc, 256:256 + QW],
                        start=(dc == 0), stop=(dc == NDC - 1),
                    )
            for hc in range(NHC):
                for dc in range(NDC):
                    nc.tensor.matmul(
                        xcp[:, hc, QW:W],
                        d2[:, dc, 128 * hc:128 * (hc + 1)],
                        d2[:, dc, 256:256 + LPT],
                        start=(dc == 0), stop=(dc == NDC - 1),
                    )

            # ---- cast to fp16 (split across V and S so neither serializes)
            xcat = basisp.tile([128, NHC, W], fp16, tag="xcat")
            nc.vector.tensor_copy(xcat[:, 0], xcp[:, 0, :W])
            nc.scalar.activation(xcat[:, 1], xcp[:, 1, :W], AF.Copy)

            sc = [ppS.tile([128, 2, LPs[s]], f32, tag=f"sc{s}", name=f"sc{s}")
                  for s in (0, 1)]

            # ---- basis slots ----
            for r, (wq, sq, wk, sk, c) in enumerate(SLOTS):
                u = uwp.tile([128, NHC, W], fp16, tag="u", name=f"u{r}")
                nc.vector._custom_dve(
                    RR, out=u[:, :, 0:QW], in0=xcat[:, :, 0:QW],
                    s0=wq / TWO_PI, s1=sq / TWO_PI, imm2=MAGIC)
                nc.vector._custom_dve(
                    RR, out=u[:, :, QW:W], in0=xcat[:, :, QW:W],
                    s0=wk / TWO_PI, s1=sk / TWO_PI, imm2=MAGIC)
                atom = atwp.tile([128, NHC, W], fp16, tag="at", name=f"at{r}")
                nc.scalar.activation(
                    atom[:].rearrange("p a b -> p (a b)"),
                    u[:].rearrange("p a b -> p (a b)"),
                    AF.Sin, scale=TWO_PI)
                vt = vtwp.tile([128, NHC, LPT], fp16, tag="vt", name=f"vt{r}")
                for hc in range(NHC):
                    nc.gpsimd.tensor_scalar(
                        vt[:, hc], atom[:, hc, QW:W], wv[:, hc:hc + 1],
                        float(c), ALU.mult, ALU.mult)
                for s in (0, 1):
                    for qc in range(2):
                        for hc in range(NHC):
                            nc.tensor.matmul(
                                sc[s][:, qc, :],
                                atom[:, hc, 256 * s + 128 * qc:
                                     256 * s + 128 * qc + 128],
                                vt[:, hc, OFS[s]:OFS[s] + LPs[s]],
                                start=(r == 0 and qc == 0 and hc == 0),
                                stop=False,
                            )

            # ---- mask pseudo-slot closes accumulation ----
            for s in (0, 1):
                for qc in range(2):
                    nc.tensor.matmul(
                        sc[s][:, qc, :], ones16[:],
                        maskv[:, OFS[s]:OFS[s] + LPs[s]],
                        start=False, stop=(qc == 1),
                    )

            # ---- softmax + attn@V epilogue (s streams interleaved) ----
            p_ts, rsums, rinvs, pTs, outps = {}, {}, {}, {}, {}
            for s in (0, 1):
                p_ts[s] = epip.tile([128, 2, LPs[s]], bf16, tag=f"pt{s}",
                                    name=f"pt{s}")
                rsums[s] = epip.tile([128, 2], f32, tag=f"rs{s}", name=f"rs{s}")
                for qc in range(2):
                    nc.scalar.activation(
                        p_ts[s][:, qc, :], sc[s][:, qc, :], AF.Exp,
                        accum_out=rsums[s][:, qc:qc + 1])
            for s in (0, 1):
                rinvs[s] = epip.tile([128, 2], f32, tag=f"ri{s}", name=f"ri{s}")
                nc.vector.reciprocal(rinvs[s][:], rsums[s][:])
            ncp = 0
            for s in (0, 1):
                pTs[s] = epip.tile([128, NKC[s], 2, 128], bf16, tag=f"pT{s}",
                                   name=f"pT{s}")
                for qc in range(2):
                    for kc in range(NKC[s]):
                        kw_ = KCW[s][kc]
                        tp = ppT.tile([128, 128], bf16, tag="tp")
                        nc.tensor.transpose(
                            tp[:kw_, :128],
                            p_ts[s][:, qc, 128 * kc:128 * kc + kw_],
                            ident_bf[:, :])
                        if ncp % 2 == 0:
                            nc.vector.tensor_copy(pTs[s][:kw_, kc, qc],
                                                  tp[:kw_, :128])
                        else:
                            nc.scalar.copy(pTs[s][:kw_, kc, qc],
                                           tp[:kw_, :128])
                        ncp += 1
            for s in (0, 1):
                outp = ppS.tile([128, 2, DV], f32, tag=f"sc{s}", name=f"op{s}")
                outps[s] = outp
                for qc in range(2):
                    for kc in range(NKC[s]):
                        kw_ = KCW[s][kc]
                        nc.tensor.matmul(
                            outp[:, qc, :], pTs[s][:kw_, kc, qc],
                            vals[:kw_, 2 * s + kc, :],
                            start=(qc == 0 and kc == 0),
                            stop=(qc == 1 and kc == NKC[s] - 1),
                        )
            for s in (0, 1):
                out_sb = epip.tile([128, 2, DV], bf16, tag=f"ob{s}",
                                   name=f"ob{s}")
                for qc in range(2):
                    if qc == 0:
                        nc.vector.tensor_scalar(
                            out_sb[:, qc, :], outps[s][:, qc, :],
                            rinvs[s][:, qc:qc + 1], None, ALU.mult)
                    else:
                        nc.scalar.mul(out_sb[:, qc, :], outps[s][:, qc, :],
                                      rinvs[s][:, qc:qc + 1])
                    nc.sync.dma_start(
                        out_d.ap()[s, 128 * qc:128 * (qc + 1), :],
                        out_sb[:, qc, :])

    nc.compile()
    return nc


_GRAPH_CACHE = {}


def _get_graph(LP0, LP1):
    key = (LP0, LP1)
    if key not in _GRAPH_CACHE:
        _GRAPH_CACHE[key] = _build_graph(LP0, LP1)
    return _GRAPH_CACHE[key]


def kernel(queries, keys, values, valid_lens, W_q, W_k, w_v):
    from concourse import bass_utils

    queries = np.asarray(queries, dtype=np.float32)
    keys = np.asarray(keys, dtype=np.float32)
    values = np.asarray(values, dtype=np.float32)
    W_q = np.asarray(W_q, dtype=np.float32)
    W_k = np.asarray(W_k, dtype=np.float32)
    w_v = np.asarray(w_v, dtype=np.float32)
    vl = np.asarray(valid_lens).astype(np.int64)

    B = queries.shape[0]
    assert B == 2 * _NCORES

    # pair batches: sort desc, pair i with (B-1-i)
    order = np.argsort(-vl, kind="stable")
    pairs = [(int(order[i]), int(order[B - 1 - i])) for i in range(_NCORES)]
    lv = lambda b: int(min(NQ, max(1, vl[b])))
    LP0 = max(-(-lv(b0) // 8) * 8 for b0, b1 in pairs)
    LP1 = max(-(-lv(b1) // 8) * 8 for b0, b1 in pairs)
    LPT = LP0 + LP1
    QW = 2 * NQ

    nc = _get_graph(LP0, LP1)

    def t128(x, rows):  # [rows, 256] -> [128, NDC, rows]
        xt = np.ascontiguousarray(x[:rows].T)          # [256, rows]
        return xt.reshape(NDC, 128, rows).transpose(1, 0, 2)

    Wq_c = W_q.reshape(NDC, 128, H).transpose(1, 0, 2)   # [128, NDC, H]
    Wk_c = W_k.reshape(NDC, 128, H).transpose(1, 0, 2)
    wvp = np.ascontiguousarray(w_v.reshape(NHC, 128).T).astype(np.float32)

    in_maps = []
    for b0, b1 in pairs:
        d1 = np.empty((128, NDC, 256 + QW), np.float32)
        d1[:, :, :256] = Wq_c
        d1[:, :, 256:512] = t128(queries[b0], NQ)
        d1[:, :, 512:768] = t128(queries[b1], NQ)
        d2 = np.empty((128, NDC, 256 + LPT), np.float32)
        d2[:, :, :256] = Wk_c
        d2[:, :, 256:256 + LP0] = t128(keys[b0], LP0)
        d2[:, :, 256 + LP0:] = t128(keys[b1], LP1)
        d3 = np.zeros((128, 4 * DV + LPT), np.float32)
        for s, (b, LPs) in enumerate(((b0, LP0), (b1, LP1))):
            for kc in range(-(-LPs // 128)):
                kw_ = min(128, LPs - 128 * kc)
                d3[:kw_, DV * (2 * s + kc):DV * (2 * s + kc) + DV] = \
                    values[b, 128 * kc:128 * kc + kw_, :]
        mk = np.zeros((128, LPT), np.float32)
        mk[:, :LP0][:, np.arange(LP0) >= lv(b0)] = -8000.0
        mk[:, LP0:][:, np.arange(LP1) >= lv(b1)] = -8000.0
        d3[:, 4 * DV:] = mk
        in_maps.append({
            "wq_qT": np.ascontiguousarray(d1).astype(FP16),
            "wk_kT": np.ascontiguousarray(d2).astype(FP16),
            "vm": d3.astype(FP16),
            "wvp": wvp,
        })

    trace = os.environ.get("BASS_KERNEL_TRACE") == "1"
    if trace:
        _register_ntff_hook()
    res = bass_utils.run_bass_kernel_spmd(
        nc, in_maps, core_ids=list(range(_NCORES)), trace=trace)
    kernel.last_results = res

    out = np.empty((B, NQ, DV), dtype=np.float32)
    for j, (b0, b1) in enumerate(pairs):
        o = np.asarray(res.results[j]["out"]).astype(np.float32)
        out[b0] = o[0]
        out[b1] = o[1]
    return out


# revision 7
# speedup vs baseline: 1.1527x; 1.0471x over previous
"""Additive attention (tanh-score) via separable sin-basis, TRN2 x8.

scores[b,q,k] = sum_h w_v[h] * tanh(qp[b,q,h] + kp[b,k,h])
              ~ sum_h w_v[h] * [ h(a) + sum_r c_r sin(w_r a + s_r)sin(w'_r b + s'_r) ]

Pure-a term is softmax-invariant => dropped. Each sin atom is evaluated as
  u = t - round(t),  t = (w x + s) / 2pi     (ONE fused custom DVE op,
                                              fp32 magic-constant rounding)
  atom = Sin(2pi * u)                        (ACT, scale/bias identical for
                                              every atom => q|k concatenated
                                              into one ACT call per slot)
The wv*c fold of the k-side runs on GpSimd (otherwise idle). Scores
accumulate in PSUM across slots; a mask pseudo-slot (ones lhsT x mask rows)
closes the groups. Softmax: fused-exp ACT with row-sum accumulator, PE
transposes, attn @ values on PE, 1/rowsum applied on the output tiles.

Sharding: 2 batches per core (big+small valid_len paired), baked LP0/LP1.
"""

import os
import numpy as np
import ml_dtypes

_NCORES = 8
BF16 = ml_dtypes.bfloat16
FP16 = np.float16

# (w_q, s_q, w_k, s_k, c): tanh(a+b) ~ sum c*sin(w_q a + s_q)*sin(w_k b + s_k)
SLOTS = [
    (-2.5000000000, 0.2873262163, -2.5000000000, -1.7679537402, 0.0454728641),
    (1.5184027148, -3.2031135619, -1.5172863264, -1.6228271159, 0.1612171955),
    (-0.4400441419, 1.5829518017, -0.4404289683, -3.1577059831, 1.1875448050),
    (-2.4691279788, -1.3872222109, 2.4692423942, 0.2646449817, -0.0516913971),
    (1.3705130957, -1.6155418902, 1.3700050256, 0.0533663718, -0.2331078125),
    (-0.7178178262, -3.1153805677, -0.7182280272, 1.5504013584, 0.5396772381),
]

NQ = 256
D = 256
H = 256
DV = 256
NDC = D // 128
NHC = H // 128
MAGIC = 12582912.0  # 1.5 * 2**23: fp32 add/sub rounds to nearest integer
TWO_PI = 6.283185307179586


def _register_ntff_hook():
    import sys, types
    try:
        from antenv.axon_hooks import get_axon_ntff_profile_hook  # noqa: F401
        return
    except ImportError:
        pass
    try:
        import trn_agent_boot.trn_boot as tb
        mod = types.ModuleType("antenv.axon_hooks")
        hook = tb._ntff_profile_via_ctypes("/opt/axon/libaxon_pjrt.so")
        mod.get_axon_ntff_profile_hook = lambda: hook
        mod.set_axon_ntff_profile_hook = lambda h: None
        sys.modules["antenv.axon_hooks"] = mod
    except Exception:
        pass


def _register_rr_op():
    """Custom DVE op: u = t - round(t), t = Src0*C0 + C1 (C2 = MAGIC)."""
    import concourse.dve_ops as dops
    from concourse.dve_spec import Spec, Src0, C0, C1, C2, lower, _has_src1
    from concourse.dve_uop import DveOpSpec

    for o in dops.OPS:
        if o.name == "SIN_RR_ANT":
            return o
    t = Src0 * C0 + C1
    n = (t + C2) - C2
    spec = Spec(
        body=t - n,
        reference=lambda in0, in1, s0, s1, imm2: (
            lambda tt: (tt - (np.float32(np.float32(tt) + np.float32(imm2))
                             - np.float32(imm2))).astype(np.float32)
        )(np.float32(in0) * np.float32(s0) + np.float32(s1)),
    )
    row = dops._CUSTOM_DVE_ROW_BASE + len(dops.OPS)
    assert row < 0x20
    shas = {}
    for ver in ("v3", "v4"):
        uops = lower(spec, ver=ver)
        s = DveOpSpec(name="SIN_RR_ANT", opcode=row, uops=uops,
                      rd1_en=_has_src1(spec))
        shas[ver] = s.sha(ver)
    op = dops.DveOp("SIN_RR_ANT", spec, subdim=False, uops_sha=shas)
    dops.OPS.append(op)
    dops._SUB_OPCODE_FOR_NAME[op.name] = row
    dops.CUSTOM_DVE_SPECS[op.name] = spec
    return op


def _build_graph(LP0, LP1):
    import concourse.bass as bass
    import concourse.tile as tile
    from concourse import bacc, mybir, masks

    RR = _register_rr_op()

    f32 = mybir.dt.float32
    bf16 = mybir.dt.bfloat16
    fp16 = mybir.dt.float16
    AF = mybir.ActivationFunctionType
    ALU = mybir.AluOpType
    PSUM = bass.MemorySpace.PSUM

    LPT = LP0 + LP1
    LPs = (LP0, LP1)
    OFS = (0, LP0)
    NKC = ((LP0 + 127) // 128, (LP1 + 127) // 128)
    KCW = [[min(128, LPs[s] - 128 * c) for c in range(NKC[s])] for s in (0, 1)]
    QW = 2 * NQ                  # 512 q-cols (both batch slots)
    W = QW + LPT                 # per-hc projection width

    nc = bacc.Bacc("TRN2", target_bir_lowering=False, debug=False,
                   num_devices=_NCORES)

    d1_d = nc.dram_tensor("wq_qT", (128, NDC, 256 + QW), fp16,
                          kind="ExternalInput")
    d2_d = nc.dram_tensor("wk_kT", (128, NDC, 256 + LPT), fp16,
                          kind="ExternalInput")
    d3_d = nc.dram_tensor("vm", (128, 4 * DV + LPT), fp16,
                          kind="ExternalInput")
    wv_d = nc.dram_tensor("wvp", (128, NHC), f32, kind="ExternalInput")
    out_d = nc.dram_tensor("out", (2, NQ, DV), bf16, kind="ExternalOutput")

    with tile.TileContext(nc) as tc:
        with (
            tc.tile_pool(name="const", bufs=1) as constp,
            tc.tile_pool(name="basis", bufs=1) as basisp,
            tc.tile_pool(name="uw", bufs=2) as uwp,
            tc.tile_pool(name="atw", bufs=2) as atwp,
            tc.tile_pool(name="vtw", bufs=2) as vtwp,
            tc.tile_pool(name="epi", bufs=1) as epip,
            tc.tile_pool(name="ppX", bufs=1, space=PSUM) as ppX,
            tc.tile_pool(name="ppS", bufs=1, space=PSUM) as ppS,
            tc.tile_pool(name="ppT", bufs=2, space=PSUM) as ppT,
        ):
            # ---- inputs (DMA order = need order) ----
            d1 = constp.tile([128, NDC, 256 + QW], fp16)
            nc.sync.dma_start(d1[:], d1_d.ap())
            d2 = constp.tile([128, NDC, 256 + LPT], fp16)
            nc.sync.dma_start(d2[:], d2_d.ap())
            d3 = constp.tile([128, 4 * DV + LPT], fp16)
            nc.sync.dma_start(d3[:], d3_d.ap())
            wv = constp.tile([128, NHC], f32)
            nc.sync.dma_start(wv[:], wv_d.ap())
            vals = d3[:, :4 * DV].rearrange("p (c v) -> p c v", c=4)
            maskv = d3[:, 4 * DV:]

            identf = constp.tile([128, 128], f32)
            masks.make_identity(nc, identf[:])
            ident_bf = constp.tile([128, 128], bf16)
            nc.vector.tensor_copy(ident_bf[:], identf[:])
            ones16 = constp.tile([128, 128], fp16)
            nc.vector.memset(ones16[:], 1.0)
            # dummy Sin on 1 col: forces the trig table set to load NOW
            # (overlapped with input DMA) so the later Copy-cast + slot Sins
            # trigger no further ACT_TABLE_LOAD until the epilogue's Exp.
            dum = constp.tile([128, 1], fp16)
            nc.scalar.activation(dum[:], ones16[:, 0:1], AF.Sin, scale=0.1)

            # ---- projections into PSUM: per hc [q-block 512 | k-block LPT]
            xcp = ppX.tile([128, NHC, 1024], f32, tag="xcp")
            for hc in range(NHC):
                for dc in range(NDC):
                    nc.tensor.matmul(
                        xcp[:, hc, 0:QW],
                        d1[:, dc, 128 * hc:128 * (hc + 1)],
                        d1[:, dc, 256:256 + QW],
                        start=(dc == 0), stop=(dc == NDC - 1),
                    )
            for hc in range(NHC):
                for dc in range(NDC):
                    nc.tensor.matmul(
                        xcp[:, hc, QW:W],
                        d2[:, dc, 128 * hc:128 * (hc + 1)],
                        d2[:, dc, 256:256 + LPT],
                        start=(dc == 0), stop=(dc == NDC - 1),
                    )

            # ---- cast to fp16, split by SIDE: q-half on V as soon as the
            # q projections close (overlaps the k projections); k-half on S.
            xcat = basisp.tile([128, NHC, W], fp16, tag="xcat")
            nc.vector.tensor_copy(xcat[:, :, 0:QW], xcp[:, :, 0:QW])
            nc.scalar.activation(xcat[:, :, QW:W], xcp[:, :, QW:W], AF.Copy)

            sc = [ppS.tile([128, 2, LPs[s]], f32, tag=f"sc{s}", name=f"sc{s}")
                  for s in (0, 1)]

            # ---- basis slots (emit slot0's q-RR before any k-RR so the
            # in-order V stream starts on the cast_q output immediately) ----
            for r, (wq, sq, wk, sk, c) in enumerate(SLOTS):
                u = uwp.tile([128, NHC, W], fp16, tag="u", name=f"u{r}")
                nc.vector._custom_dve(
                    RR, out=u[:, :, 0:QW], in0=xcat[:, :, 0:QW],
                    s0=wq / TWO_PI, s1=sq / TWO_PI, imm2=MAGIC)
                nc.vector._custom_dve(
                    RR, out=u[:, :, QW:W], in0=xcat[:, :, QW:W],
                    s0=wk / TWO_PI, s1=sk / TWO_PI, imm2=MAGIC)
                atom = atwp.tile([128, NHC, W], fp16, tag="at", name=f"at{r}")
                nc.scalar.activation(
                    atom[:].rearrange("p a b -> p (a b)"),
                    u[:].rearrange("p a b -> p (a b)"),
                    AF.Sin, scale=TWO_PI)
                vt = vtwp.tile([128, NHC, LPT], fp16, tag="vt", name=f"vt{r}")
                for hc in range(NHC):
                    nc.gpsimd.tensor_scalar(
                        vt[:, hc], atom[:, hc, QW:W], wv[:, hc:hc + 1],
                        float(c), ALU.mult, ALU.mult)
                for s in (0, 1):
                    for qc in range(2):
                        for hc in range(NHC):
                            nc.tensor.matmul(
                                sc[s][:, qc, :],
                                atom[:, hc, 256 * s + 128 * qc:
                                     256 * s + 128 * qc + 128],
                                vt[:, hc, OFS[s]:OFS[s] + LPs[s]],
                                start=(r == 0 and qc == 0 and hc == 0),
                                stop=False,
                            )

            # ---- mask pseudo-slot closes accumulation ----
            for s in (0, 1):
                for qc in range(2):
                    nc.tensor.matmul(
                        sc[s][:, qc, :], ones16[:],
                        maskv[:, OFS[s]:OFS[s] + LPs[s]],
                        start=False, stop=(qc == 1),
                    )

            # ---- softmax + attn@V epilogue (s streams interleaved) ----
            p_ts, rsums, rinvs, pTs, outps = {}, {}, {}, {}, {}
            for s in (0, 1):
                p_ts[s] = epip.tile([128, 2, LPs[s]], bf16, tag=f"pt{s}",
                                    name=f"pt{s}")
                rsums[s] = epip.tile([128, 2], f32, tag=f"rs{s}", name=f"rs{s}")
                for qc in range(2):
                    nc.scalar.activation(
                        p_ts[s][:, qc, :], sc[s][:, qc, :], AF.Exp,
                        accum_out=rsums[s][:, qc:qc + 1])
            for s in (0, 1):
                rinvs[s] = epip.tile([128, 2], f32, tag=f"ri{s}", name=f"ri{s}")
                nc.vector.reciprocal(rinvs[s][:], rsums[s][:])
            ncp = 0
            for s in (0, 1):
                pTs[s] = epip.tile([128, NKC[s], 2, 128], bf16, tag=f"pT{s}",
                                   name=f"pT{s}")
                for qc in range(2):
                    for kc in range(NKC[s]):
                        kw_ = KCW[s][kc]
                        tp = ppT.tile([128, 128], bf16, tag="tp")
                        nc.tensor.transpose(
                            tp[:kw_, :128],
                            p_ts[s][:, qc, 128 * kc:128 * kc + kw_],
                            ident_bf[:, :])
                        if ncp % 2 == 0:
                            nc.vector.tensor_copy(pTs[s][:kw_, kc, qc],
                                                  tp[:kw_, :128])
                        else:
                            nc.scalar.copy(pTs[s][:kw_, kc, qc],
                                           tp[:kw_, :128])
                        ncp += 1
            for s in (0, 1):
                outp = ppS.tile([128, 2, DV], f32, tag=f"sc{s}", name=f"op{s}")
                outps[s] = outp
                for qc in range(2):
                    for kc in range(NKC[s]):
                        kw_ = KCW[s][kc]
                        nc.tensor.matmul(
                            outp[:, qc, :], pTs[s][:kw_, kc, qc],
                            vals[:kw_, 2 * s + kc, :],
                            start=(qc == 0 and kc == 0),
                            stop=(qc == 1 and kc == NKC[s] - 1),
                        )
            for s in (0, 1):
                out_sb = epip.tile([128, 2, DV], bf16, tag=f"ob{s}",
                                   name=f"ob{s}")
                for qc in range(2):
                    nc.vector.tensor_scalar(
                        out_sb[:, qc, :], outps[s][:, qc, :],
                        rinvs[s][:, qc:qc + 1], None, ALU.mult)
                    nc.sync.dma_start(
                        out_d.ap()[s, 128 * qc:128 * (qc + 1), :],
                        out_sb[:, qc, :])

    nc.compile()
    return nc


_GRAPH_CACHE = {}


def _get_graph(LP0, LP1):
    key = (LP0, LP1)
    if key not in _GRAPH_CACHE:
        _GRAPH_CACHE[key] = _build_graph(LP0, LP1)
    return _GRAPH_CACHE[key]


def kernel(queries, keys, values, valid_lens, W_q, W_k, w_v):
    from concourse import bass_utils

    queries = np.asarray(queries, dtype=np.float32)
    keys = np.asarray(keys, dtype=np.float32)
    values = np.asarray(values, dtype=np.float32)
    W_q = np.asarray(W_q, dtype=np.float32)
    W_k = np.asarray(W_k, dtype=np.float32)
    w_v = np.asarray(w_v, dtype=np.float32)
    vl = np.asarray(valid_lens).astype(np.int64)

    B = queries.shape[0]
    assert B == 2 * _NCORES

    # pair batches: sort desc, pair i with (B-1-i)
    order = np.argsort(-vl, kind="stable")
    pairs = [(int(order[i]), int(order[B - 1 - i])) for i in range(_NCORES)]
    lv = lambda b: int(min(NQ, max(1, vl[b])))
    LP0 = max(-(-lv(b0) // 8) * 8 for b0, b1 in pairs)
    LP1 = max(-(-lv(b1) // 8) * 8 for b0, b1 in pairs)
    LPT = LP0 + LP1
    QW = 2 * NQ

    nc = _get_graph(LP0, LP1)

    def t128(x, rows):  # [rows, 256] -> [128, NDC, rows]
        xt = np.ascontiguousarray(x[:rows].T)          # [256, rows]
        return xt.reshape(NDC, 128, rows).transpose(1, 0, 2)

    Wq_c = W_q.reshape(NDC, 128, H).transpose(1, 0, 2)   # [128, NDC, H]
    Wk_c = W_k.reshape(NDC, 128, H).transpose(1, 0, 2)
    wvp = np.ascontiguousarray(w_v.reshape(NHC, 128).T).astype(np.float32)

    in_maps = []
    for b0, b1 in pairs:
        d1 = np.empty((128, NDC, 256 + QW), np.float32)
        d1[:, :, :256] = Wq_c
        d1[:, :, 256:512] = t128(queries[b0], NQ)
        d1[:, :, 512:768] = t128(queries[b1], NQ)
        d2 = np.empty((128, NDC, 256 + LPT), np.float32)
        d2[:, :, :256] = Wk_c
        d2[:, :, 256:256 + LP0] = t128(keys[b0], LP0)
        d2[:, :, 256 + LP0:] = t128(keys[b1], LP1)
        d3 = np.zeros((128, 4 * DV + LPT), np.float32)
        for s, (b, LPs) in enumerate(((b0, LP0), (b1, LP1))):
            for kc in range(-(-LPs // 128)):
                kw_ = min(128, LPs - 128 * kc)
                d3[:kw_, DV * (2 * s + kc):DV * (2 * s + kc) + DV] = \
                    values[b, 128 * kc:128 * kc + kw_, :]
        mk = np.zeros((128, LPT), np.float32)
        mk[:, :LP0][:, np.arange(LP0) >= lv(b0)] = -8000.0
        mk[:, LP0:][:, np.arange(LP1) >= lv(b1)] = -8000.0
        d3[:, 4 * DV:] = mk
        in_maps.append({
            "wq_qT": np.ascontiguousarray(d1).astype(FP16),
            "wk_kT": np.ascontiguousarray(d2).astype(FP16),
            "vm": d3.astype(FP16),
            "wvp": wvp,
        })

    trace = os.environ.get("BASS_KERNEL_TRACE") == "1"
    if trace:
        _register_ntff_hook()
    res = bass_utils.run_bass_kernel_spmd(
        nc, in_maps, core_ids=list(range(_NCORES)), trace=trace)
    kernel.last_results = res

    out = np.empty((B, NQ, DV), dtype=np.float32)
    for j, (b0, b1) in enumerate(pairs):
        o = np.asarray(res.results[j]["out"]).astype(np.float32)
        out[b0] = o[0]
        out[b1] = o[1]
    return out


# revision 8
# speedup vs baseline: 1.1714x; 1.0162x over previous
"""Additive attention (tanh-score) via separable sin-basis, TRN2 x8.

scores[b,q,k] = sum_h w_v[h] * tanh(qp[b,q,h] + kp[b,k,h])
              ~ sum_h w_v[h] * [ h(a) + sum_r c_r sin(w_r a + s_r)sin(w'_r b + s'_r) ]

Pure-a term is softmax-invariant => dropped. Each sin atom is evaluated as
  u = t - round(t),  t = (w x + s) / 2pi     (ONE fused custom DVE op,
                                              fp32 magic-constant rounding)
  atom = Sin(2pi * u)                        (ACT, scale/bias identical for
                                              every atom => q|k concatenated
                                              into one ACT call per slot)
The wv*c fold of the k-side runs on GpSimd (otherwise idle). Scores
accumulate in PSUM across slots; a mask pseudo-slot (ones lhsT x mask rows)
closes the groups. Softmax: fused-exp ACT with row-sum accumulator, PE
transposes, attn @ values on PE, 1/rowsum applied on the output tiles.

Sharding: 2 batches per core (big+small valid_len paired), baked LP0/LP1.
"""

import os
import numpy as np
import ml_dtypes

_NCORES = 8
BF16 = ml_dtypes.bfloat16
FP16 = np.float16

# (w_q, s_q, w_k, s_k, c): tanh(a+b) ~ sum c*sin(w_q a + s_q)*sin(w_k b + s_k)
SLOTS = [
    (-2.5000000000, 0.2873262163, -2.5000000000, -1.7679537402, 0.0454728641),
    (1.5184027148, -3.2031135619, -1.5172863264, -1.6228271159, 0.1612171955),
    (-0.4400441419, 1.5829518017, -0.4404289683, -3.1577059831, 1.1875448050),
    (-2.4691279788, -1.3872222109, 2.4692423942, 0.2646449817, -0.0516913971),
    (1.3705130957, -1.6155418902, 1.3700050256, 0.0533663718, -0.2331078125),
    (-0.7178178262, -3.1153805677, -0.7182280272, 1.5504013584, 0.5396772381),
]

NQ = 256
D = 256
H = 256
DV = 256
NDC = D // 128
NHC = H // 128
MAGIC = 12582912.0  # 1.5 * 2**23: fp32 add/sub rounds to nearest integer
TWO_PI = 6.283185307179586


def _register_ntff_hook():
    import sys, types
    try:
        from antenv.axon_hooks import get_axon_ntff_profile_hook  # noqa: F401
        return
    except ImportError:
        pass
    try:
        import trn_agent_boot.trn_boot as tb
        mod = types.ModuleType("antenv.axon_hooks")
        hook = tb._ntff_profile_via_ctypes("/opt/axon/libaxon_pjrt.so")
        mod.get_axon_ntff_profile_hook = lambda: hook
        mod.set_axon_ntff_profile_hook = lambda h: None
        sys.modules["antenv.axon_hooks"] = mod
    except Exception:
        pass


def _register_rr_op():
    """Custom DVE op: u = t - round(t), t = Src0*C0 + C1 (C2 = MAGIC)."""
    import concourse.dve_ops as dops
    from concourse.dve_spec import Spec, Src0, C0, C1, C2, lower, _has_src1
    from concourse.dve_uop import DveOpSpec

    for o in dops.OPS:
        if o.name == "SIN_RR_ANT":
            return o
    t = Src0 * C0 + C1
    n = (t + C2) - C2
    spec = Spec(
        body=t - n,
        reference=lambda in0, in1, s0, s1, imm2: (
            lambda tt: (tt - (np.float32(np.float32(tt) + np.float32(imm2))
                             - np.float32(imm2))).astype(np.float32)
        )(np.float32(in0) * np.float32(s0) + np.float32(s1)),
    )
    row = dops._CUSTOM_DVE_ROW_BASE + len(dops.OPS)
    assert row < 0x20
    shas = {}
    for ver in ("v3", "v4"):
        uops = lower(spec, ver=ver)
        s = DveOpSpec(name="SIN_RR_ANT", opcode=row, uops=uops,
                      rd1_en=_has_src1(spec))
        shas[ver] = s.sha(ver)
    op = dops.DveOp("SIN_RR_ANT", spec, subdim=False, uops_sha=shas)
    dops.OPS.append(op)
    dops._SUB_OPCODE_FOR_NAME[op.name] = row
    dops.CUSTOM_DVE_SPECS[op.name] = spec
    return op


def _build_graph(LP0, LP1):
    import concourse.bass as bass
    import concourse.tile as tile
    from concourse import bacc, mybir, masks

    RR = _register_rr_op()

    f32 = mybir.dt.float32
    bf16 = mybir.dt.bfloat16
    fp16 = mybir.dt.float16
    AF = mybir.ActivationFunctionType
    ALU = mybir.AluOpType
    PSUM = bass.MemorySpace.PSUM

    LPT = LP0 + LP1
    LPs = (LP0, LP1)
    OFS = (0, LP0)
    NKC = ((LP0 + 127) // 128, (LP1 + 127) // 128)
    KCW = [[min(128, LPs[s] - 128 * c) for c in range(NKC[s])] for s in (0, 1)]
    QW = 2 * NQ                  # 512 q-cols (both batch slots)
    W = QW + LPT                 # per-hc projection width

    nc = bacc.Bacc("TRN2", target_bir_lowering=False, debug=False,
                   num_devices=_NCORES)

    d1_d = nc.dram_tensor("wq_qT", (128, NDC, 256 + QW), fp16,
                          kind="ExternalInput")
    d2_d = nc.dram_tensor("wk_kT", (128, NDC, 256 + LPT), fp16,
                          kind="ExternalInput")
    d3_d = nc.dram_tensor("vm", (128, 4 * DV + LPT), fp16,
                          kind="ExternalInput")
    wv_d = nc.dram_tensor("wvp", (128, NHC), f32, kind="ExternalInput")
    out_d = nc.dram_tensor("out", (2, NQ, DV), bf16, kind="ExternalOutput")

    with tile.TileContext(nc) as tc:
        with (
            tc.tile_pool(name="const", bufs=1) as constp,
            tc.tile_pool(name="basis", bufs=1) as basisp,
            tc.tile_pool(name="uw", bufs=2) as uwp,
            tc.tile_pool(name="atw", bufs=2) as atwp,
            tc.tile_pool(name="vtw", bufs=2) as vtwp,
            tc.tile_pool(name="epi", bufs=1) as epip,
            tc.tile_pool(name="ppX", bufs=1, space=PSUM) as ppX,
            tc.tile_pool(name="ppS", bufs=1, space=PSUM) as ppS,
            tc.tile_pool(name="ppT", bufs=2, space=PSUM) as ppT,
        ):
            # ---- inputs (DMA order = need order) ----
            d1 = constp.tile([128, NDC, 256 + QW], fp16)
            nc.sync.dma_start(d1[:], d1_d.ap())
            d2 = constp.tile([128, NDC, 256 + LPT], fp16)
            nc.sync.dma_start(d2[:], d2_d.ap())
            d3 = constp.tile([128, 4 * DV + LPT], fp16)
            nc.sync.dma_start(d3[:], d3_d.ap())
            wv = constp.tile([128, NHC], f32)
            nc.sync.dma_start(wv[:], wv_d.ap())
            vals = d3[:, :4 * DV].rearrange("p (c v) -> p c v", c=4)
            maskv = d3[:, 4 * DV:]

            identf = constp.tile([128, 128], f32)
            masks.make_identity(nc, identf[:])
            ident_bf = constp.tile([128, 128], bf16)
            nc.vector.tensor_copy(ident_bf[:], identf[:])
            ones16 = constp.tile([128, 128], fp16)
            nc.vector.memset(ones16[:], 1.0)
            # dummy Sin on 1 col: forces the trig table set to load NOW
            # (overlapped with input DMA) so the later Copy-cast + slot Sins
            # trigger no further ACT_TABLE_LOAD until the epilogue's Exp.
            dum = constp.tile([128, 1], fp16)
            nc.scalar.activation(dum[:], ones16[:, 0:1], AF.Sin, scale=0.1)

            # ---- projections into PSUM: per hc [q-block 512 | k-block LPT]
            xcp = ppX.tile([128, NHC, 1024], f32, tag="xcp")
            for hc in range(NHC):
                for dc in range(NDC):
                    nc.tensor.matmul(
                        xcp[:, hc, 0:QW],
                        d1[:, dc, 128 * hc:128 * (hc + 1)],
                        d1[:, dc, 256:256 + QW],
                        start=(dc == 0), stop=(dc == NDC - 1),
                    )
            for hc in range(NHC):
                for dc in range(NDC):
                    nc.tensor.matmul(
                        xcp[:, hc, QW:W],
                        d2[:, dc, 128 * hc:128 * (hc + 1)],
                        d2[:, dc, 256:256 + LPT],
                        start=(dc == 0), stop=(dc == NDC - 1),
                    )

            sc = [ppS.tile([128, 2, LPs[s]], f32, tag=f"sc{s}", name=f"sc{s}")
                  for s in (0, 1)]

            # ---- basis slots. The RR ops read the projection PSUM directly
            # (custom DVE ops run 1x either way) so there is no fp16 cast and
            # slot0's q-side RR starts the moment the q projections close.
            # The mask pseudo-slot is interleaved into the last slot's PE
            # stream: each (s,qc) group is closed right after its last hc MM.
            RLAST = len(SLOTS) - 1
            for r, (wq, sq, wk, sk, c) in enumerate(SLOTS):
                u = uwp.tile([128, NHC, W], fp16, tag="u", name=f"u{r}")
                nc.vector._custom_dve(
                    RR, out=u[:, :, 0:QW], in0=xcp[:, :, 0:QW],
                    s0=wq / TWO_PI, s1=sq / TWO_PI, imm2=MAGIC)
                nc.vector._custom_dve(
                    RR, out=u[:, :, QW:W], in0=xcp[:, :, QW:W],
                    s0=wk / TWO_PI, s1=sk / TWO_PI, imm2=MAGIC)
                atom = atwp.tile([128, NHC, W], fp16, tag="at", name=f"at{r}")
                nc.scalar.activation(
                    atom[:].rearrange("p a b -> p (a b)"),
                    u[:].rearrange("p a b -> p (a b)"),
                    AF.Sin, scale=TWO_PI)
                vt = vtwp.tile([128, NHC, LPT], fp16, tag="vt", name=f"vt{r}")
                for hc in range(NHC):
                    nc.gpsimd.tensor_scalar(
                        vt[:, hc], atom[:, hc, QW:W], wv[:, hc:hc + 1],
                        float(c), ALU.mult, ALU.mult)
                for s in (0, 1):
                    for qc in range(2):
                        for hc in range(NHC):
                            nc.tensor.matmul(
                                sc[s][:, qc, :],
                                atom[:, hc, 256 * s + 128 * qc:
                                     256 * s + 128 * qc + 128],
                                vt[:, hc, OFS[s]:OFS[s] + LPs[s]],
                                start=(r == 0 and qc == 0 and hc == 0),
                                stop=False,
                            )
                        if r == RLAST:
                            nc.tensor.matmul(
                                sc[s][:, qc, :], ones16[:],
                                maskv[:, OFS[s]:OFS[s] + LPs[s]],
                                start=False, stop=True,
                            )

            # ---- softmax + attn@V epilogue (s streams interleaved) ----
            p_ts, rsums, rinvs, pTs, outps = {}, {}, {}, {}, {}
            for s in (0, 1):
                p_ts[s] = epip.tile([128, 2, LPs[s]], bf16, tag=f"pt{s}",
                                    name=f"pt{s}")
                rsums[s] = epip.tile([128, 2], f32, tag=f"rs{s}", name=f"rs{s}")
                for qc in range(2):
                    nc.scalar.activation(
                        p_ts[s][:, qc, :], sc[s][:, qc, :], AF.Exp,
                        accum_out=rsums[s][:, qc:qc + 1])
            for s in (0, 1):
                rinvs[s] = epip.tile([128, 2], f32, tag=f"ri{s}", name=f"ri{s}")
                nc.vector.reciprocal(rinvs[s][:], rsums[s][:])
            ncp = 0
            for s in (0, 1):
                pTs[s] = epip.tile([128, NKC[s], 2, 128], bf16, tag=f"pT{s}",
                                   name=f"pT{s}")
                for qc in range(2):
                    for kc in range(NKC[s]):
                        kw_ = KCW[s][kc]
                        tp = ppT.tile([128, 128], bf16, tag="tp")
                        nc.tensor.transpose(
                            tp[:kw_, :128],
                            p_ts[s][:, qc, 128 * kc:128 * kc + kw_],
                            ident_bf[:, :])
                        if ncp % 2 == 0:
                            nc.vector.tensor_copy(pTs[s][:kw_, kc, qc],
                                                  tp[:kw_, :128])
                        else:
                            nc.scalar.copy(pTs[s][:kw_, kc, qc],
                                           tp[:kw_, :128])
                        ncp += 1
            for s in (0, 1):
                outp = ppS.tile([128, 2, DV], f32, tag=f"sc{s}", name=f"op{s}")
                outps[s] = outp
                for qc in range(2):
                    for kc in range(NKC[s]):
                        kw_ = KCW[s][kc]
                        nc.tensor.matmul(
                            outp[:, qc, :], pTs[s][:kw_, kc, qc],
                            vals[:kw_, 2 * s + kc, :],
                            start=(qc == 0 and kc == 0),
                            stop=(qc == 1 and kc == NKC[s] - 1),
                        )
            for s in (0, 1):
                out_sb = epip.tile([128, 2, DV], bf16, tag=f"ob{s}",
                                   name=f"ob{s}")
                for qc in range(2):
                    nc.vector.tensor_scalar(
                        out_sb[:, qc, :], outps[s][:, qc, :],
                        rinvs[s][:, qc:qc + 1], None, ALU.mult)
                    nc.sync.dma_start(
                        out_d.ap()[s, 128 * qc:128 * (qc + 1), :],
                        out_sb[:, qc, :])

    nc.compile()
    return nc


_GRAPH_CACHE = {}


def _get_graph(LP0, LP1):
    key = (LP0, LP1)
    if key not in _GRAPH_CACHE:
        _GRAPH_CACHE[key] = _build_graph(LP0, LP1)
    return _GRAPH_CACHE[key]


def kernel(queries, keys, values, valid_lens, W_q, W_k, w_v):
    from concourse import bass_utils

    queries = np.asarray(queries, dtype=np.float32)
    keys = np.asarray(keys, dtype=np.float32)
    values = np.asarray(values, dtype=np.float32)
    W_q = np.asarray(W_q, dtype=np.float32)
    W_k = np.asarray(W_k, dtype=np.float32)
    w_v = np.asarray(w_v, dtype=np.float32)
    vl = np.asarray(valid_lens).astype(np.int64)

    B = queries.shape[0]
    assert B == 2 * _NCORES

    # pair batches: sort desc, pair i with (B-1-i)
    order = np.argsort(-vl, kind="stable")
    pairs = [(int(order[i]), int(order[B - 1 - i])) for i in range(_NCORES)]
    lv = lambda b: int(min(NQ, max(1, vl[b])))
    LP0 = max(-(-lv(b0) // 8) * 8 for b0, b1 in pairs)
    LP1 = max(-(-lv(b1) // 8) * 8 for b0, b1 in pairs)
    LPT = LP0 + LP1
    QW = 2 * NQ

    nc = _get_graph(LP0, LP1)

    def t128(x, rows):  # [rows, 256] -> [128, NDC, rows]
        xt = np.ascontiguousarray(x[:rows].T)          # [256, rows]
        return xt.reshape(NDC, 128, rows).transpose(1, 0, 2)

    Wq_c = W_q.reshape(NDC, 128, H).transpose(1, 0, 2)   # [128, NDC, H]
    Wk_c = W_k.reshape(NDC, 128, H).transpose(1, 0, 2)
    wvp = np.ascontiguousarray(w_v.reshape(NHC, 128).T).astype(np.float32)

    in_maps = []
    for b0, b1 in pairs:
        d1 = np.empty((128, NDC, 256 + QW), np.float32)
        d1[:, :, :256] = Wq_c
        d1[:, :, 256:512] = t128(queries[b0], NQ)
        d1[:, :, 512:768] = t128(queries[b1], NQ)
        d2 = np.empty((128, NDC, 256 + LPT), np.float32)
        d2[:, :, :256] = Wk_c
        d2[:, :, 256:256 + LP0] = t128(keys[b0], LP0)
        d2[:, :, 256 + LP0:] = t128(keys[b1], LP1)
        d3 = np.zeros((128, 4 * DV + LPT), np.float32)
        for s, (b, LPs) in enumerate(((b0, LP0), (b1, LP1))):
            for kc in range(-(-LPs // 128)):
                kw_ = min(128, LPs - 128 * kc)
                d3[:kw_, DV * (2 * s + kc):DV * (2 * s + kc) + DV] = \
                    values[b, 128 * kc:128 * kc + kw_, :]
        mk = np.zeros((128, LPT), np.float32)
        mk[:, :LP0][:, np.arange(LP0) >= lv(b0)] = -8000.0
        mk[:, LP0:][:, np.arange(LP1) >= lv(b1)] = -8000.0
        d3[:, 4 * DV:] = mk
        in_maps.append({
            "wq_qT": np.ascontiguousarray(d1).astype(FP16),
            "wk_kT": np.ascontiguousarray(d2).astype(FP16),
            "vm": d3.astype(FP16),
            "wvp": wvp,
        })

    trace = os.environ.get("BASS_KERNEL_TRACE") == "1"
    if trace:
        _register_ntff_hook()
    res = bass_utils.run_bass_kernel_spmd(
        nc, in_maps, core_ids=list(range(_NCORES)), trace=trace)
    kernel.last_results = res

    out = np.empty((B, NQ, DV), dtype=np.float32)
    for j, (b0, b1) in enumerate(pairs):
        o = np.asarray(res.results[j]["out"]).astype(np.float32)
        out[b0] = o[0]
        out[b1] = o[1]
    return out
